# revision 7
# baseline (speedup 1.0000x reference)
"""CogView EMA VQ quantizer — Trainium2 Bass kernel (8-core data-parallel).

Contract: kernel(**inputs) takes FULL inputs (x [8,64,64,256] f32,
embed [256,8192] f32, cluster_size [8192] f32, embed_avg [256,8192] f32)
and returns the reference 6-tuple:
  (quantize_st, diff, embed_ind, embed_new, cluster_size_new, embed_avg_new)

Device strategy (per core, 4096 tokens in 32 tiles of 128):
  - approx scores s = x@e - ||e||^2/2 via fp16 matmuls (fp32 PSUM accumulate),
    bias folded in as a K=1 matmul row
  - two offset-pair max folds (8192 -> 2048 "quads"), Max8 + MaxIndex -> top-2
    quads -> 8 candidate codes/token
  - dma_gather candidate embedding rows (fp32), exact rescore
    d = ||x||^2 + sum(e*(e-2x)) on GPSIMD+DVE, argmin w/ smallest-code tiebreak
  - quantize gather by final index; quantize_st = x + (q - x)
  - per-tile duplicate merge (selection matmul) + dma_scatter_add into a
    [8193,256] slab (dummy row 8192 for dup rows) -> embed_sum partials
Host: shard/gather, histogram via bincount, EMA update, and exact jnp-CPU
re-computation of dist rows for near-tie tokens (gap < 1e-3) to match the
reference argmin bit-for-bit.
"""

import numpy as np

DIM = 256
NE = 8192
DECAY = 0.99
EPS = 1e-05
N_CORES = 8
P = 128
TPC = 4096          # tokens per core
NT = TPC // P       # tiles per core (32)
NCHUNK = 16         # 512-wide code chunks
CHUNK = 512
GAP_THRESH = 1e-3

_STATE = {}


def _build(n_tiles=NT):
    import concourse.bass as bass
    import concourse.mybir as mybir
    import concourse.tile as tile
    from concourse import bacc
    from concourse.masks import make_identity

    dt = mybir.dt
    AO = mybir.AluOpType
    ntok = n_tiles * P

    nc = bacc.Bacc(trn_type="TRN2", target_bir_lowering=False, debug=False)

    xpart = nc.dram_tensor("xpart", [ntok, DIM], dt.float32, kind="ExternalInput").ap()
    embed16 = nc.dram_tensor("embed16", [DIM, NE], dt.float16, kind="ExternalInput").ap()
    negh16 = nc.dram_tensor("negh16", [1, NE], dt.float16, kind="ExternalInput").ap()
    etab = nc.dram_tensor("etab", [NE, DIM], dt.float32, kind="ExternalInput").ap()

    aux = nc.dram_tensor("aux", [ntok, 4], dt.float32, kind="ExternalOutput").ap()
    qst_o = nc.dram_tensor("qst", [ntok, DIM], dt.float32, kind="ExternalOutput").ap()
    slab = nc.dram_tensor("slab", [NE + 1, DIM], dt.float32, kind="ExternalOutput").ap()

    with tile.TileContext(nc) as tc:
        with (
            tc.tile_pool(name="const", bufs=1) as cpool,
            tc.tile_pool(name="work", bufs=2) as wpool,
            tc.tile_pool(name="xin", bufs=3) as xpool,
            tc.tile_pool(name="psum", bufs=3, space="PSUM") as pspool,
            tc.tile_pool(name="psmg", bufs=2, space="PSUM") as mgpool,
            tc.tile_pool(name="dram", bufs=2, space="DRAM") as dpool,
        ):
            # ---- constants ----
            emb_sb = cpool.tile([P, 2, NE], dt.float16)
            nc.sync.dma_start(emb_sb[:], embed16.rearrange("(o p) c -> p o c", p=P))
            negh_sb = cpool.tile([1, NE], dt.float16)
            nc.sync.dma_start(negh_sb[:], negh16[:])
            ones1 = cpool.tile([1, P], dt.float16)
            nc.vector.memset(ones1[:], 1.0)
            ones128 = cpool.tile([P, 1], dt.float16)
            nc.vector.memset(ones128[:], 1.0)
            ident = cpool.tile([P, P], dt.float32)
            make_identity(nc, ident[:])
            # strict lower-triangular ones (L[p, c] = c > p), fp16
            coli = cpool.tile([P, P], dt.int16)
            nc.gpsimd.iota(coli[:], pattern=[[1, P]], base=0, channel_multiplier=0)
            rowi = cpool.tile([P, 1], dt.int16)
            nc.gpsimd.iota(rowi[:], pattern=[[0, 1]], base=0, channel_multiplier=1)
            rowf = cpool.tile([P, 1], dt.float32)
            nc.vector.tensor_copy(rowf[:], rowi[:])
            ltri = cpool.tile([P, P], dt.float16)
            nc.vector.tensor_scalar(ltri[:], coli[:], rowf[:, 0:1], None, op0=AO.is_gt)
            # candidate offsets row [128, 4] = (0, 128, 256, 384)
            offs = cpool.tile([P, 4], dt.uint16)
            for j in range(4):
                nc.vector.memset(offs[:, j:j + 1], j * 128)
            # j-index row [128, 8] = 1..8
            jrow = cpool.tile([P, 8], dt.float32)
            for j in range(8):
                nc.vector.memset(jrow[:, j:j + 1], float(j + 1))
            # zero the slab
            zsb = cpool.tile([P, 2048], dt.float32)
            nc.vector.memset(zsb[:], 0.0)
            flat_slab = slab.rearrange("a b -> (a b)")
            total = (NE + 1) * DIM
            off = 0
            while off < total:
                n = min(P * 2048, total - off)
                rows = n // 2048
                if rows >= 1:
                    nc.sync.dma_start(
                        flat_slab[off:off + rows * 2048].rearrange("(p f) -> p f", p=rows),
                        zsb[:rows, :],
                    )
                    off += rows * 2048
                else:
                    nc.sync.dma_start(
                        flat_slab[off:total].rearrange("(p f) -> p f", p=1),
                        zsb[:1, : total - off],
                    )
                    off = total

            for t in range(n_tiles):
                # ---- load & cast x ----
                x32 = xpool.tile([P, DIM], dt.float32, tag="x32")
                nc.sync.dma_start(x32[:], xpart[t * P:(t + 1) * P, :])
                x16 = xpool.tile([P, DIM], dt.float16, tag="x16")
                nc.vector.tensor_copy(x16[:], x32[:])
                xT = xpool.tile([P, 2, P], dt.float16, tag="xT")
                for k in range(2):
                    nc.sync.dma_start_transpose(xT[:, k, :], x16[:, k * P:(k + 1) * P])

                # ---- dist matmuls + bias row; evac to fp16 scores ----
                s16 = wpool.tile([P, NCHUNK, CHUNK], dt.float16, tag="s16")
                for g in range(8):
                    ps = pspool.tile([P, 2 * CHUNK], dt.float32, tag="dist")
                    for jj in range(2):
                        j = 2 * g + jj
                        sl = ps[:, jj * CHUNK:(jj + 1) * CHUNK]
                        nc.tensor.matmul(sl, xT[:, 0, :], emb_sb[:, 0, j * CHUNK:(j + 1) * CHUNK], start=True, stop=False)
                        nc.tensor.matmul(sl, xT[:, 1, :], emb_sb[:, 1, j * CHUNK:(j + 1) * CHUNK], start=False, stop=False)
                        nc.tensor.matmul(sl, ones1[:], negh_sb[:, j * CHUNK:(j + 1) * CHUNK], start=False, stop=True)
                    nc.any.tensor_copy(
                        s16[:, 2 * g:2 * g + 2, :].rearrange("p a b -> p (a b)"),
                        ps[:],
                    )

                # ---- folds: pair (k, k+256) then (k, k+128) ----
                s16v = s16[:].rearrange("p c (two k) -> p c two k", two=2)
                f1 = wpool.tile([P, NCHUNK, 256], dt.float16, tag="f1")
                nc.vector.tensor_tensor(f1[:], s16v[:, :, 0, :], s16v[:, :, 1, :], op=AO.max)
                f1v = f1[:].rearrange("p c (two k) -> p c two k", two=2)
                f2 = wpool.tile([P, NCHUNK, P], dt.float16, tag="f2")
                nc.vector.tensor_tensor(f2[:], f1v[:, :, 0, :], f1v[:, :, 1, :], op=AO.max)

                # ---- top-8 quads -> top-2 -> 8 candidates ----
                mx = wpool.tile([P, 8], dt.float16, tag="mx")
                f2flat = f2[:].rearrange("p a b -> p (a b)")
                nc.vector.max(mx[:], f2flat)
                mi = wpool.tile([P, 8], dt.uint16, tag="mi")
                nc.vector.max_index(mi[:], mx[:], f2flat)

                # base = (q >> 7) << 9; k = q & 127; cand = base + k + {0,128,256,384}
                bq = wpool.tile([P, 2], dt.uint16, tag="bq")
                nc.vector.tensor_scalar(bq[:], mi[:, 0:2], 7, None, op0=AO.logical_shift_right)
                nc.vector.tensor_scalar(bq[:], bq[:], 9, None, op0=AO.logical_shift_left)
                kq = wpool.tile([P, 2], dt.uint16, tag="kq")
                nc.vector.tensor_scalar(kq[:], mi[:, 0:2], 127, None, op0=AO.bitwise_and)
                nc.vector.tensor_tensor(bq[:], bq[:], kq[:], op=AO.add)
                candu = wpool.tile([P, 2, 4], dt.uint16, tag="candu")
                nc.vector.tensor_tensor(
                    candu[:], bq[:, :, None].to_broadcast([P, 2, 4]),
                    offs[:, None, :].to_broadcast([P, 2, 4]), op=AO.add)
                cufl = candu[:].rearrange("p a b -> p (a b)")
                candf = wpool.tile([P, 8], dt.float32, tag="candf")
                nc.vector.tensor_copy(candf[:], cufl)
                cfl = candf[:]
                ci16 = wpool.tile([P, 8], dt.int16, tag="ci16")
                nc.vector.tensor_copy(ci16[:], cufl)

                # ---- wrap candidate idxs via DRAM bounce ----
                cb = dpool.tile([P, 8], dt.int16)
                nc.sync.dma_start(cb[:], ci16[:])
                widx = wpool.tile([P, 64], dt.int16, tag="widx")
                wsrc = cb[:].rearrange("(c0 p) c1 -> p c1 c0", p=16)
                for grp in range(8):
                    nc.sync.dma_start(
                        widx[grp * 16:(grp + 1) * 16, :].rearrange(
                            "p (c1 c0) -> p c1 c0", c1=8), wsrc)

                # ---- gather candidate rows [P, 8, 256] fp32 ----
                G = wpool.tile([P, 8, DIM], dt.float32, tag="G")
                nc.gpsimd.dma_gather(
                    out_ap=G[:], in_ap=etab[:], idxs_ap=widx[:],
                    num_idxs=8 * P, num_idxs_reg=8 * P, elem_size=DIM)

                # ---- exact rescore: d = T1 + sum(e*(e-2x)) ----
                x2 = wpool.tile([P, DIM], dt.float32, tag="x2")
                nc.vector.tensor_scalar(x2[:], x32[:], 2.0, None, op0=AO.mult)
                e2x = wpool.tile([P, 8, DIM], dt.float32, tag="e2x")
                nc.gpsimd.tensor_tensor(
                    e2x[:], G[:], x2[:, None, :].to_broadcast([P, 8, DIM]), op=AO.subtract)
                nc.gpsimd.tensor_tensor(e2x[:], G[:], e2x[:], op=AO.mult)
                dsum = wpool.tile([P, 8], dt.float32, tag="dsum")
                nc.vector.tensor_reduce(dsum[:], e2x[:], axis=mybir.AxisListType.X, op=AO.add)
                # T1 = sum(x^2) via ACT square-accumulate
                xsqj = wpool.tile([P, DIM], dt.float32, tag="xsqj")
                t1 = wpool.tile([P, 1], dt.float32, tag="t1")
                nc.scalar.activation(xsqj[:], x32[:], mybir.ActivationFunctionType.Square,
                                     accum_out=t1[:])
                dfin = wpool.tile([P, 8], dt.float32, tag="dfin")
                nc.vector.tensor_scalar(dfin[:], dsum[:], t1[:, 0:1], None, op0=AO.add)

                # ---- choose argmin, smallest-code tiebreak ----
                dmin = wpool.tile([P, 1], dt.float32, tag="dmin")
                nc.vector.tensor_reduce(dmin[:], dfin[:], axis=mybir.AxisListType.X, op=AO.min)
                mask = wpool.tile([P, 8], dt.float32, tag="mask")
                nc.vector.tensor_scalar(mask[:], dfin[:], dmin[:, 0:1], None, op0=AO.is_equal)
                # crev = mask * (-cand); enc = mask*(8192-cand) = mask*8192 + crev
                crev = wpool.tile([P, 8], dt.float32, tag="crev")
                nc.vector.scalar_tensor_tensor(crev[:], cfl, -1.0, mask[:], op0=AO.mult, op1=AO.mult)
                enc = wpool.tile([P, 8], dt.float32, tag="enc")
                nc.vector.scalar_tensor_tensor(enc[:], mask[:], 8192.0, crev[:], op0=AO.mult, op1=AO.add)
                emax = wpool.tile([P, 1], dt.float32, tag="emax")
                nc.vector.tensor_reduce(emax[:], enc[:], axis=mybir.AxisListType.X, op=AO.max)
                indf = wpool.tile([P, 1], dt.float32, tag="indf")
                nc.vector.tensor_scalar(indf[:], emax[:], -1.0, None, op0=AO.mult)
                nc.vector.tensor_scalar(indf[:], indf[:], 8192.0, None, op0=AO.add)
                # d2 = min over (d + 1e9*mask)
                dbig = wpool.tile([P, 8], dt.float32, tag="dbig")
                nc.vector.scalar_tensor_tensor(dbig[:], mask[:], 1e9, dfin[:], op0=AO.mult, op1=AO.add)
                d2 = wpool.tile([P, 1], dt.float32, tag="d2")
                nc.vector.tensor_reduce(d2[:], dbig[:], axis=mybir.AxisListType.X, op=AO.min)

                # ---- scatter index (dups -> dummy row 8192) ----
                # first-occurrence mask via selection matrix
                ind_bc = indf[:, 0:1].to_broadcast([P, P])
                mgps = mgpool.tile([P, CHUNK], dt.float32, tag="mg")
                nc.tensor.transpose(mgps[:, 0:P], ind_bc, ident[:])
                indT = wpool.tile([P, P], dt.float32, tag="indT")
                nc.any.tensor_copy(indT[:], mgps[:, 0:P])
                sel16 = wpool.tile([P, P], dt.float16, tag="sel16")
                nc.vector.tensor_tensor(sel16[:], ind_bc, indT[:], op=AO.is_equal)
                lsel = wpool.tile([P, P], dt.float16, tag="lsel")
                nc.vector.tensor_tensor(lsel[:], sel16[:], ltri[:], op=AO.mult)
                nc.tensor.matmul(mgps[:, 384:385], lsel[:], ones128[:], start=True, stop=True)
                fmask = wpool.tile([P, 1], dt.float32, tag="fmask")
                nc.vector.tensor_scalar(fmask[:], mgps[:, 384:385], 0.0, None, op0=AO.is_equal)
                # merged sums of duplicate rows
                nc.tensor.matmul(mgps[:, P:P + DIM], sel16[:], x16[:], start=True, stop=True)
                scat_in = wpool.tile([P, 1, DIM], dt.float32, tag="scatin")
                nc.scalar.activation(scat_in[:, 0, :], mgps[:, P:P + DIM],
                                     mybir.ActivationFunctionType.Copy, scale=fmask[:, 0:1])
                # sidx = 8192 + fmask*(ind-8192)
                sidxf = wpool.tile([P, 1], dt.float32, tag="sidxf")
                nc.vector.tensor_scalar(sidxf[:], indf[:], 8192.0, None, op0=AO.subtract)
                nc.vector.tensor_tensor(sidxf[:], sidxf[:], fmask[:], op=AO.mult)
                nc.vector.tensor_scalar(sidxf[:], sidxf[:], 8192.0, None, op0=AO.add)
                # pack ind & sidx into [P, 2] int16, bounce, wrap
                isx = wpool.tile([P, 2], dt.int16, tag="isx")
                nc.vector.tensor_copy(isx[:, 0:1], indf[:])
                nc.vector.tensor_copy(isx[:, 1:2], sidxf[:])
                ib = dpool.tile([P, 2], dt.int16)
                nc.sync.dma_start(ib[:], isx[:])
                wib = wpool.tile([P, 16], dt.int16, tag="wib")
                wibsrc = ib[:].rearrange("(c0 p) t -> p t c0", p=16)
                for grp in range(8):
                    nc.sync.dma_start(
                        wib[grp * 16:(grp + 1) * 16, :].rearrange(
                            "p (t c0) -> p t c0", t=2), wibsrc)

                # ---- quantize gather + straight-through ----
                q = wpool.tile([P, 1, DIM], dt.float32, tag="q")
                nc.gpsimd.dma_gather(
                    out_ap=q[:], in_ap=etab[:], idxs_ap=wib[:, 0:8],
                    num_idxs=P, num_idxs_reg=P, elem_size=DIM)
                qd = wpool.tile([P, DIM], dt.float32, tag="qd")
                nc.vector.tensor_tensor(qd[:], q[:, 0, :], x32[:], op=AO.subtract)
                nc.vector.tensor_tensor(qd[:], x32[:], qd[:], op=AO.add)
                nc.sync.dma_start(qst_o[t * P:(t + 1) * P, :], qd[:])

                # ---- scatter-add ----
                nc.gpsimd.dma_scatter_add(
                    out_ap=slab[:], in_ap=scat_in[:], idxs_ap=wib[:, 8:16],
                    num_idxs=P, num_idxs_reg=P, elem_size=DIM)

                # ---- aux out ----
                auxsb = wpool.tile([P, 4], dt.float32, tag="auxsb")
                nc.vector.tensor_copy(auxsb[:, 0:1], indf[:])
                nc.vector.tensor_copy(auxsb[:, 1:2], dmin[:])
                nc.vector.tensor_copy(auxsb[:, 2:3], d2[:])
                nc.vector.tensor_copy(auxsb[:, 3:4], t1[:])
                nc.sync.dma_start(aux[t * P:(t + 1) * P, :], auxsb[:])

    nc.compile()
    return nc


def _get_nc(n_tiles=NT):
    key = ("nc", n_tiles)
    if key not in _STATE:
        _STATE[key] = _build(n_tiles)
    return _STATE[key]


def _host_prep(x, embed):
    flat = np.ascontiguousarray(x.reshape(-1, DIM)).astype(np.float32, copy=False)
    embed = np.asarray(embed, np.float32)
    embed16 = embed.astype(np.float16)
    h = (embed.astype(np.float32) ** 2).sum(0, dtype=np.float32)
    negh16 = (-(h / 2.0)).astype(np.float16)[None, :]
    etab = np.ascontiguousarray(embed.T)
    return flat, embed16, negh16, etab, h


def kernel(x, embed, cluster_size, embed_avg):
    from concourse import bass_utils

    x = np.asarray(x, np.float32)
    embed = np.asarray(embed, np.float32)
    cluster_size = np.asarray(cluster_size, np.float32)
    embed_avg = np.asarray(embed_avg, np.float32)

    flat, embed16, negh16, etab, h = _host_prep(x, embed)
    N = flat.shape[0]
    assert N == N_CORES * TPC

    nc = _get_nc()
    in_maps = []
    for c in range(N_CORES):
        in_maps.append({
            "xpart": flat[c * TPC:(c + 1) * TPC],
            "embed16": embed16,
            "negh16": negh16,
            "etab": etab,
        })
    res = bass_utils.run_bass_kernel_spmd(nc, in_maps, core_ids=list(range(N_CORES)))

    aux = np.concatenate([res.results[c]["aux"] for c in range(N_CORES)], axis=0)
    qst = np.concatenate([res.results[c]["qst"] for c in range(N_CORES)], axis=0)
    embed_sum = np.zeros((NE, DIM), np.float32)
    for c in range(N_CORES):
        embed_sum += res.results[c]["slab"][:NE]
    embed_sum = embed_sum.T.copy()          # [256, 8192]

    ind = aux[:, 0].astype(np.int64)
    d1 = aux[:, 1].astype(np.float64)
    d2 = aux[:, 2]

    # host disambiguation: recompute exact reference dist rows for near ties
    gap = d2 - aux[:, 1]
    fix = np.nonzero(gap < GAP_THRESH)[0]
    if fix.size:
        import jax
        import jax.numpy as jnp
        cpu = jax.devices("cpu")[0]
        with jax.default_device(cpu):
            f = jnp.asarray(flat[fix])
            e = jnp.asarray(embed)
            dist = (jnp.sum(f * f, axis=1, keepdims=True) - 2.0 * (f @ e)
                    + jnp.sum(e * e, axis=0, keepdims=True))
            new_ind = np.asarray(jnp.argmin(dist, axis=1)).astype(np.int64)
            dist = np.asarray(dist)
        d1[fix] = dist[np.arange(fix.size), new_ind]
        changed_mask = new_ind != ind[fix]
        for tk, nw in zip(fix[changed_mask], new_ind[changed_mask]):
            od = int(ind[tk])
            ind[tk] = nw
            q_new = embed[:, nw]
            qst[tk] = flat[tk] + (q_new - flat[tk])
            embed_sum[:, od] -= flat[tk]
            embed_sum[:, nw] += flat[tk]

    # EMA / outputs (fp32 host math mirroring the reference)
    counts = np.bincount(ind, minlength=NE).astype(np.float32)
    cluster_size_new = (cluster_size * np.float32(DECAY)
                        + np.float32(1.0 - DECAY) * counts).astype(np.float32)
    embed_avg_new = (embed_avg * np.float32(DECAY)
                     + np.float32(1.0 - DECAY) * embed_sum).astype(np.float32)
    n = np.float32(cluster_size_new.sum(dtype=np.float64))
    cs = (cluster_size_new + np.float32(EPS)) / (n + np.float32(NE * EPS)) * n
    embed_new = (embed_avg_new / cs[None, :]).astype(np.float32)

    diff = np.float32(d1.sum() / (N * DIM))
    quantize_st = qst.reshape(x.shape).astype(np.float32)
    embed_ind = ind.astype(np.int32).reshape(x.shape[:-1])

    return (quantize_st, diff, embed_ind, embed_new,
            cluster_size_new, embed_avg_new)


# revision 13
# speedup vs baseline: 1.8327x; 1.8327x over previous
"""CogView EMA VQ quantizer — Trainium2 Bass kernel (8-core data-parallel).

kernel(**inputs) takes FULL inputs (x [8,64,64,256] f32, embed [256,8192] f32,
cluster_size [8192] f32, embed_avg [256,8192] f32) and returns the reference
6-tuple (quantize_st, diff, embed_ind, embed_new, cluster_size_new,
embed_avg_new).

Device (per core, 4096 tokens as 8 super-tiles x 4 tiles x 128 tokens):
  - approx scores s = x@e - ||e||^2/2 via fp16 matmuls (fp32 PSUM), bias as a
    K=1 matmul row
  - two offset-pair max folds (8192 -> 2048 quads), Max8 + MaxIndex -> top-2
    quads -> 8 candidate codes/token
  - per-candidate indirect-DMA gather of fp32 embedding rows, exact rescore
    d = ||x||^2 + sum(e*(e-2x)), argmin with smallest-code tiebreak
  - quantize via indirect gather of the final index; qst = x + (q - x)
  - per-tile duplicate merge (selection matmul, dups -> dummy row) +
    indirect scatter-accumulate into 2 alternating [8193,256] slabs,
    merged on device at the end
Host: shard/gather, bincount histogram, EMA update, and exact jnp-CPU
recompute of dist rows for near-tie tokens (gap < 1e-3) to match the
reference argmin bit-for-bit.
"""

import numpy as np

DIM = 256
NE = 8192
DECAY = 0.99
EPS = 1e-05
N_CORES = 8
P = 128
TPC = 4096          # tokens per core
ST = 4              # tiles per super-tile
NST = TPC // (P * ST)   # super-tiles per core (8)
NCHUNK = 16
CHUNK = 512
GAP_THRESH = 1e-3

_STATE = {}


def _build(nst=NST):
    import concourse.bass as bass
    import concourse.mybir as mybir
    import concourse.tile as tile
    from concourse import bacc
    from concourse.masks import make_identity

    dt = mybir.dt
    AO = mybir.AluOpType
    AF = mybir.ActivationFunctionType
    AX = mybir.AxisListType
    IOff = bass.IndirectOffsetOnAxis
    ntok = nst * ST * P

    nc = bacc.Bacc(trn_type="TRN2", target_bir_lowering=False, debug=False)

    xpart = nc.dram_tensor("xpart", [ntok, DIM], dt.float32, kind="ExternalInput").ap()
    embed16 = nc.dram_tensor("embed16", [DIM, NE], dt.float16, kind="ExternalInput").ap()
    negh16 = nc.dram_tensor("negh16", [1, NE], dt.float16, kind="ExternalInput").ap()
    etab = nc.dram_tensor("etab", [NE, DIM], dt.float32, kind="ExternalInput").ap()

    aux = nc.dram_tensor("aux", [ntok, 4], dt.float32, kind="ExternalOutput").ap()
    qst_o = nc.dram_tensor("qst", [ntok, DIM], dt.float32, kind="ExternalOutput").ap()
    slab0 = nc.dram_tensor("slab0", [NE + 1, DIM], dt.float32, kind="ExternalOutput").ap()
    slab1 = nc.dram_tensor("slab1", [NE + 1, DIM], dt.float32, kind="Internal").ap()

    with tile.TileContext(nc) as tc:
        with (
            tc.tile_pool(name="const", bufs=1) as cpool,
            tc.tile_pool(name="work", bufs=2) as wpool,
            tc.tile_pool(name="big", bufs=2) as bpool,
            tc.tile_pool(name="xin", bufs=2) as xpool,
            tc.tile_pool(name="psum", bufs=3, space="PSUM") as pspool,
            tc.tile_pool(name="psmg", bufs=2, space="PSUM") as mgpool,
        ):
            # ---- constants ----
            emb_sb = cpool.tile([P, 2, NE], dt.float16)
            nc.sync.dma_start(emb_sb[:], embed16.rearrange("(o p) c -> p o c", p=P))
            negh_sb = cpool.tile([1, NE], dt.float16)
            nc.sync.dma_start(negh_sb[:], negh16[:])
            ones1 = cpool.tile([1, P], dt.float16)
            nc.vector.memset(ones1[:], 1.0)
            ones128 = cpool.tile([P, 1], dt.float16)
            nc.vector.memset(ones128[:], 1.0)
            ident = cpool.tile([P, P], dt.float32)
            make_identity(nc, ident[:])
            coli = cpool.tile([P, P], dt.int16)
            nc.gpsimd.iota(coli[:], pattern=[[1, P]], base=0, channel_multiplier=0)
            rowi = cpool.tile([P, 1], dt.int16)
            nc.gpsimd.iota(rowi[:], pattern=[[0, 1]], base=0, channel_multiplier=1)
            rowf = cpool.tile([P, 1], dt.float32)
            nc.vector.tensor_copy(rowf[:], rowi[:])
            ltri = cpool.tile([P, P], dt.float16)
            nc.vector.tensor_scalar(ltri[:], coli[:], rowf[:, 0:1], None, op0=AO.is_gt)
            offs = cpool.tile([P, 4], dt.uint16)
            for j in range(4):
                nc.vector.memset(offs[:, j:j + 1], j * 128)
            # zero both slabs
            zsb = cpool.tile([P, 512], dt.float32)
            nc.vector.memset(zsb[:], 0.0)
            for slab in (slab0, slab1):
                flat_slab = slab.rearrange("a b -> (a b)")
                total = (NE + 1) * DIM
                off = 0
                while off < total:
                    n = min(P * 512, total - off)
                    rows = max(n // 512, 1)
                    w = min(512, total - off)
                    nc.sync.dma_start(
                        flat_slab[off:off + rows * w].rearrange("(p f) -> p f", p=rows),
                        zsb[:rows, :w])
                    off += rows * w

            for st in range(nst):
                base = st * ST * P
                # ---- batched x load / cast ----
                xld = xpool.tile([P, ST, DIM], dt.float32, tag="xld")
                nc.sync.dma_start(
                    xld[:], xpart[base:base + ST * P, :].rearrange("(s p) d -> p s d", p=P))
                x16b = xpool.tile([P, ST, DIM], dt.float16, tag="x16b")
                nc.vector.tensor_copy(x16b[:], xld[:])
                xTs = []
                for s in range(ST):
                    xT = xpool.tile([P, 2, P], dt.float16, tag=f"xT{s}")
                    for k in range(2):
                        nc.sync.dma_start_transpose(xT[:, k, :], x16b[:, s, k * P:(k + 1) * P])
                    xTs.append(xT)

                mxST = wpool.tile([P, ST, 8], dt.float16, tag="mxST")
                miST = wpool.tile([P, ST, 8], dt.uint16, tag="miST")
                for s in range(ST):
                    xT = xTs[s]
                    # ---- dist matmuls + bias; evac to fp16 ----
                    s16 = bpool.tile([P, NCHUNK, CHUNK], dt.float16, tag="s16")
                    for g in range(8):
                        ps = pspool.tile([P, 2 * CHUNK], dt.float32, tag="dist")
                        for jj in range(2):
                            j = 2 * g + jj
                            sl = ps[:, jj * CHUNK:(jj + 1) * CHUNK]
                            nc.tensor.matmul(sl, xT[:, 0, :], emb_sb[:, 0, j * CHUNK:(j + 1) * CHUNK], start=True, stop=False)
                            nc.tensor.matmul(sl, xT[:, 1, :], emb_sb[:, 1, j * CHUNK:(j + 1) * CHUNK], start=False, stop=False)
                            nc.tensor.matmul(sl, ones1[:], negh_sb[:, j * CHUNK:(j + 1) * CHUNK], start=False, stop=True)
                        nc.any.tensor_copy(
                            s16[:, 2 * g:2 * g + 2, :].rearrange("p a b -> p (a b)"), ps[:])
                    # ---- folds ----
                    s16v = s16[:].rearrange("p c (two k) -> p c two k", two=2)
                    f1 = bpool.tile([P, NCHUNK, 256], dt.float16, tag="f1")
                    nc.vector.tensor_tensor(f1[:], s16v[:, :, 0, :], s16v[:, :, 1, :], op=AO.max)
                    f1v = f1[:].rearrange("p c (two k) -> p c two k", two=2)
                    # fold2 into the (now dead) first quarter of s16 - contiguous
                    f2s = s16[:, 0:4, :].rearrange("p a b -> p (a b)")
                    nc.vector.tensor_tensor(f2s, f1v[:, :, 0, :], f1v[:, :, 1, :], op=AO.max)
                    nc.vector.max(mxST[:, s, :], f2s)
                    nc.vector.max_index(miST[:, s, :], mxST[:, s, :], f2s)

                # ---- batched candidate expansion ----
                bq = wpool.tile([P, ST, 2], dt.uint16, tag="bq")
                mp = miST[:, :, 0:2]
                nc.vector.tensor_scalar(bq[:], mp, 7, None, op0=AO.logical_shift_right)
                nc.vector.tensor_scalar(bq[:], bq[:], 9, None, op0=AO.logical_shift_left)
                kq = wpool.tile([P, ST, 2], dt.uint16, tag="kq")
                nc.vector.tensor_scalar(kq[:], mp, 127, None, op0=AO.bitwise_and)
                nc.vector.tensor_tensor(bq[:], bq[:], kq[:], op=AO.add)
                candu = wpool.tile([P, ST, 2, 4], dt.uint16, tag="candu")
                nc.vector.tensor_tensor(
                    candu[:], bq[:, :, :, None].to_broadcast([P, ST, 2, 4]),
                    offs[:, None, None, :].to_broadcast([P, ST, 2, 4]), op=AO.add)
                candf = wpool.tile([P, ST, 8], dt.float32, tag="candf")
                cu3 = candu[:].rearrange("p s a b -> p s (a b)")
                nc.vector.tensor_copy(candf[:], cu3)
                cand32 = wpool.tile([P, ST, 8], dt.int32, tag="cand32")
                nc.vector.tensor_copy(cand32[:], cu3)

                # ---- rescore ----
                x2 = wpool.tile([P, ST, DIM], dt.float32, tag="x2")
                nc.vector.tensor_scalar(x2[:], xld[:], 2.0, None, op0=AO.mult)
                dsumST = wpool.tile([P, ST, 8], dt.float32, tag="dsumST")
                t1ST = wpool.tile([P, ST], dt.float32, tag="t1ST")
                for s in range(ST):
                    G = bpool.tile([P, 8, DIM], dt.float32, tag="G")
                    for j in range(8):
                        nc.gpsimd.indirect_dma_start(
                            out=G[:, j, :], out_offset=None, in_=etab[:],
                            in_offset=IOff(ap=cand32[:, s, j:j + 1], axis=0))
                    e2x = bpool.tile([P, 8, DIM], dt.float32, tag="e2x")
                    nc.gpsimd.tensor_tensor(
                        e2x[:], G[:], x2[:, s, None, :].to_broadcast([P, 8, DIM]), op=AO.subtract)
                    nc.gpsimd.tensor_tensor(e2x[:], G[:], e2x[:], op=AO.mult)
                    nc.vector.tensor_reduce(dsumST[:, s, :], e2x[:], axis=AX.X, op=AO.add)
                    xsqj = wpool.tile([P, DIM], dt.float32, tag="xsqj")
                    nc.scalar.activation(xsqj[:], xld[:, s, :], AF.Square,
                                         accum_out=t1ST[:, s:s + 1])

                dfin = wpool.tile([P, ST, 8], dt.float32, tag="dfin")
                nc.vector.tensor_tensor(
                    dfin[:], dsumST[:], t1ST[:, :, None].to_broadcast([P, ST, 8]), op=AO.add)
                dmin = wpool.tile([P, ST], dt.float32, tag="dmin")
                nc.vector.tensor_reduce(dmin[:], dfin[:], axis=AX.X, op=AO.min)
                mask = wpool.tile([P, ST, 8], dt.float32, tag="mask")
                nc.vector.tensor_tensor(
                    mask[:], dfin[:], dmin[:, :, None].to_broadcast([P, ST, 8]), op=AO.is_equal)
                crev = wpool.tile([P, ST, 8], dt.float32, tag="crev")
                nc.vector.scalar_tensor_tensor(crev[:], candf[:], -1.0, mask[:], op0=AO.mult, op1=AO.mult)
                enc = wpool.tile([P, ST, 8], dt.float32, tag="enc")
                nc.vector.scalar_tensor_tensor(enc[:], mask[:], 8192.0, crev[:], op0=AO.mult, op1=AO.add)
                emax = wpool.tile([P, ST], dt.float32, tag="emax")
                nc.vector.tensor_reduce(emax[:], enc[:], axis=AX.X, op=AO.max)
                indf = wpool.tile([P, ST], dt.float32, tag="indf")
                nc.vector.tensor_scalar(indf[:], emax[:], -1.0, None, op0=AO.mult)
                nc.vector.tensor_scalar(indf[:], indf[:], 8192.0, None, op0=AO.add)
                dbig = wpool.tile([P, ST, 8], dt.float32, tag="dbig")
                nc.vector.scalar_tensor_tensor(dbig[:], mask[:], 1e9, dfin[:], op0=AO.mult, op1=AO.add)
                d2 = wpool.tile([P, ST], dt.float32, tag="d2")
                nc.vector.tensor_reduce(d2[:], dbig[:], axis=AX.X, op=AO.min)
                ind32 = wpool.tile([P, ST], dt.int32, tag="ind32")
                nc.vector.tensor_copy(ind32[:], indf[:])

                # ---- quantize gather + straight-through (batched out) ----
                qg = wpool.tile([P, ST, DIM], dt.float32, tag="qg")
                for s in range(ST):
                    nc.gpsimd.indirect_dma_start(
                        out=qg[:, s, :], out_offset=None, in_=etab[:],
                        in_offset=IOff(ap=ind32[:, s:s + 1], axis=0))
                qd = wpool.tile([P, ST, DIM], dt.float32, tag="qd")
                nc.vector.tensor_tensor(qd[:], qg[:], xld[:], op=AO.subtract)
                nc.vector.tensor_tensor(qd[:], xld[:], qd[:], op=AO.add)
                nc.sync.dma_start(
                    qst_o[base:base + ST * P, :].rearrange("(s p) d -> p s d", p=P), qd[:])

                # ---- duplicate merge + scatter ----
                fmaskST = wpool.tile([P, ST], dt.float32, tag="fmaskST")
                scat_in = wpool.tile([P, ST, DIM], dt.float32, tag="scatin")
                for s in range(ST):
                    ind_bc = indf[:, s:s + 1].to_broadcast([P, P])
                    mgps = mgpool.tile([P, CHUNK], dt.float32, tag="mg")
                    nc.tensor.transpose(mgps[:, 0:P], ind_bc, ident[:])
                    indT = wpool.tile([P, P], dt.float32, tag="indT")
                    nc.any.tensor_copy(indT[:], mgps[:, 0:P])
                    sel16 = wpool.tile([P, P], dt.float16, tag="sel16")
                    nc.vector.tensor_tensor(sel16[:], ind_bc, indT[:], op=AO.is_equal)
                    lsel = wpool.tile([P, P], dt.float16, tag="lsel")
                    nc.vector.tensor_tensor(lsel[:], sel16[:], ltri[:], op=AO.mult)
                    nc.tensor.matmul(mgps[:, 384:385], lsel[:], ones128[:], start=True, stop=True)
                    nc.vector.tensor_scalar(fmaskST[:, s:s + 1], mgps[:, 384:385], 0.0, None, op0=AO.is_equal)
                    nc.tensor.matmul(mgps[:, P:P + DIM], sel16[:], x16b[:, s, :], start=True, stop=True)
                    nc.scalar.activation(scat_in[:, s, :], mgps[:, P:P + DIM],
                                         AF.Copy, scale=fmaskST[:, s:s + 1])
                sidxf = wpool.tile([P, ST], dt.float32, tag="sidxf")
                nc.vector.tensor_scalar(sidxf[:], indf[:], 8192.0, None, op0=AO.subtract)
                nc.vector.tensor_tensor(sidxf[:], sidxf[:], fmaskST[:], op=AO.mult)
                nc.vector.tensor_scalar(sidxf[:], sidxf[:], 8192.0, None, op0=AO.add)
                sidx32 = wpool.tile([P, ST], dt.int32, tag="sidx32")
                nc.vector.tensor_copy(sidx32[:], sidxf[:])
                for s in range(ST):
                    slab = slab0 if (s % 2 == 0) else slab1
                    nc.gpsimd.indirect_dma_start(
                        out=slab[:], out_offset=IOff(ap=sidx32[:, s:s + 1], axis=0),
                        in_=scat_in[:, s, :], in_offset=None, compute_op=AO.add)

                # ---- aux out ----
                auxsb = wpool.tile([P, ST, 4], dt.float32, tag="auxsb")
                nc.vector.tensor_copy(auxsb[:, :, 0], indf[:])
                nc.vector.tensor_copy(auxsb[:, :, 1], dmin[:])
                nc.vector.tensor_copy(auxsb[:, :, 2], d2[:])
                nc.vector.tensor_copy(auxsb[:, :, 3], t1ST[:])
                nc.sync.dma_start(
                    aux[base:base + ST * P, :].rearrange("(s p) c -> p s c", p=P), auxsb[:])

            # ---- merge slab1 into slab0 via DMA accumulate ----
            s0f = slab0.rearrange("a b -> (a b)")
            s1f = slab1.rearrange("a b -> (a b)")
            blk = P * 512
            pieces = [(i * blk, P, 512) for i in range(NE * DIM // blk)]
            pieces.append((NE * DIM, 1, DIM))  # dummy row (not needed, but cheap)
            for off, rows, w in pieces:
                mt = wpool.tile([P, 512], dt.float32, tag="mrg")
                nc.sync.dma_start(
                    mt[:rows, :w],
                    s1f[off:off + rows * w].rearrange("(p f) -> p f", p=rows))
                nc.gpsimd.dma_start(
                    s0f[off:off + rows * w].rearrange("(p f) -> p f", p=rows),
                    mt[:rows, :w], accum_op=AO.add)

    nc.compile()
    return nc


def _get_nc(nst=NST):
    key = ("nc", nst)
    if key not in _STATE:
        _STATE[key] = _build(nst)
    return _STATE[key]


def _host_prep(x, embed):
    flat = np.ascontiguousarray(x.reshape(-1, DIM)).astype(np.float32, copy=False)
    embed = np.asarray(embed, np.float32)
    embed16 = embed.astype(np.float16)
    h = (embed.astype(np.float32) ** 2).sum(0, dtype=np.float32)
    negh16 = (-(h / 2.0)).astype(np.float16)[None, :]
    etab = np.ascontiguousarray(embed.T)
    return flat, embed16, negh16, etab, h


def kernel(x, embed, cluster_size, embed_avg):
    from concourse import bass_utils

    x = np.asarray(x, np.float32)
    embed = np.asarray(embed, np.float32)
    cluster_size = np.asarray(cluster_size, np.float32)
    embed_avg = np.asarray(embed_avg, np.float32)

    flat, embed16, negh16, etab, h = _host_prep(x, embed)
    N = flat.shape[0]
    assert N == N_CORES * TPC

    nc = _get_nc()
    in_maps = []
    for c in range(N_CORES):
        in_maps.append({
            "xpart": flat[c * TPC:(c + 1) * TPC],
            "embed16": embed16,
            "negh16": negh16,
            "etab": etab,
        })
    res = bass_utils.run_bass_kernel_spmd(nc, in_maps, core_ids=list(range(N_CORES)))

    aux = np.concatenate([res.results[c]["aux"] for c in range(N_CORES)], axis=0)
    qst = np.concatenate([res.results[c]["qst"] for c in range(N_CORES)], axis=0)
    embed_sum = np.zeros((NE, DIM), np.float32)
    for c in range(N_CORES):
        embed_sum += res.results[c]["slab0"][:NE]
    embed_sum = embed_sum.T.copy()          # [256, 8192]

    ind = aux[:, 0].astype(np.int64)
    d1 = aux[:, 1].astype(np.float64)
    d2 = aux[:, 2]

    gap = d2 - aux[:, 1]
    fix = np.nonzero(gap < GAP_THRESH)[0]
    if fix.size:
        import jax
        import jax.numpy as jnp
        cpu = jax.devices("cpu")[0]
        with jax.default_device(cpu):
            f = jnp.asarray(flat[fix])
            e = jnp.asarray(embed)
            dist = (jnp.sum(f * f, axis=1, keepdims=True) - 2.0 * (f @ e)
                    + jnp.sum(e * e, axis=0, keepdims=True))
            new_ind = np.asarray(jnp.argmin(dist, axis=1)).astype(np.int64)
            dist = np.asarray(dist)
        d1[fix] = dist[np.arange(fix.size), new_ind]
        changed_mask = new_ind != ind[fix]
        for tk, nw in zip(fix[changed_mask], new_ind[changed_mask]):
            od = int(ind[tk])
            ind[tk] = nw
            q_new = embed[:, nw]
            qst[tk] = flat[tk] + (q_new - flat[tk])
            embed_sum[:, od] -= flat[tk]
            embed_sum[:, nw] += flat[tk]

    counts = np.bincount(ind, minlength=NE).astype(np.float32)
    cluster_size_new = (cluster_size * np.float32(DECAY)
                        + np.float32(1.0 - DECAY) * counts).astype(np.float32)
    embed_avg_new = (embed_avg * np.float32(DECAY)
                     + np.float32(1.0 - DECAY) * embed_sum).astype(np.float32)
    n = np.float32(cluster_size_new.sum(dtype=np.float64))
    cs = (cluster_size_new + np.float32(EPS)) / (n + np.float32(NE * EPS)) * n
    embed_new = (embed_avg_new / cs[None, :]).astype(np.float32)

    diff = np.float32(d1.sum() / (N * DIM))
    quantize_st = qst.reshape(x.shape).astype(np.float32)
    embed_ind = ind.astype(np.int32).reshape(x.shape[:-1])

    return (quantize_st, diff, embed_ind, embed_new,
            cluster_size_new, embed_avg_new)


# revision 14
# speedup vs baseline: 1.8883x; 1.0303x over previous
"""CogView EMA VQ quantizer — Trainium2 Bass kernel (8-core data-parallel).

kernel(**inputs) takes FULL inputs (x [8,64,64,256] f32, embed [256,8192] f32,
cluster_size [8192] f32, embed_avg [256,8192] f32) and returns the reference
6-tuple (quantize_st, diff, embed_ind, embed_new, cluster_size_new,
embed_avg_new).

Device (per core, 4096 tokens as 8 super-tiles x 4 tiles x 128 tokens):
  - approx scores s = x@e - ||e||^2/2 via fp16 matmuls (fp32 PSUM), bias as a
    K=1 matmul row
  - two offset-pair max folds (8192 -> 2048 quads), Max8 + MaxIndex -> top-2
    quads -> 8 candidate codes/token
  - per-candidate indirect-DMA gather of fp32 embedding rows, exact rescore
    d = ||x||^2 + sum(e*(e-2x)), argmin with smallest-code tiebreak
  - quantize via indirect gather of the final index; qst = x + (q - x)
  - per-tile duplicate merge (selection matmul, dups -> dummy row) +
    indirect scatter-accumulate into 2 alternating [8193,256] slabs,
    merged on device at the end
Host: shard/gather, bincount histogram, EMA update, and exact jnp-CPU
recompute of dist rows for near-tie tokens (gap < 1e-3) to match the
reference argmin bit-for-bit.
"""

import numpy as np

DIM = 256
NE = 8192
DECAY = 0.99
EPS = 1e-05
N_CORES = 8
P = 128
TPC = 4096          # tokens per core
ST = 4              # tiles per super-tile
NST = TPC // (P * ST)   # super-tiles per core (8)
NCHUNK = 16
CHUNK = 512
GAP_THRESH = 1e-3

_STATE = {}


def _build(nst=NST):
    import concourse.bass as bass
    import concourse.mybir as mybir
    import concourse.tile as tile
    from concourse import bacc
    from concourse.masks import make_identity

    dt = mybir.dt
    AO = mybir.AluOpType
    AF = mybir.ActivationFunctionType
    AX = mybir.AxisListType
    IOff = bass.IndirectOffsetOnAxis
    ntok = nst * ST * P

    nc = bacc.Bacc(trn_type="TRN2", target_bir_lowering=False, debug=False)

    xpart = nc.dram_tensor("xpart", [ntok, DIM], dt.float32, kind="ExternalInput").ap()
    embed16 = nc.dram_tensor("embed16", [DIM, NE], dt.float16, kind="ExternalInput").ap()
    h16b_d = nc.dram_tensor("h16b", [P, NE], dt.float16, kind="ExternalInput").ap()
    etab = nc.dram_tensor("etab", [NE, DIM], dt.float32, kind="ExternalInput").ap()

    aux = nc.dram_tensor("aux", [ntok, 4], dt.float32, kind="ExternalOutput").ap()
    qst_o = nc.dram_tensor("qst", [ntok, DIM], dt.float32, kind="ExternalOutput").ap()
    slab0 = nc.dram_tensor("slab0", [NE + 1, DIM], dt.float32, kind="ExternalOutput").ap()
    slab1 = nc.dram_tensor("slab1", [NE + 1, DIM], dt.float32, kind="Internal").ap()

    with tile.TileContext(nc) as tc:
        with (
            tc.tile_pool(name="const", bufs=1) as cpool,
            tc.tile_pool(name="work", bufs=2) as wpool,
            tc.tile_pool(name="big", bufs=2) as bpool,
            tc.tile_pool(name="xin", bufs=2) as xpool,
            tc.tile_pool(name="psum", bufs=3, space="PSUM") as pspool,
            tc.tile_pool(name="psmg", bufs=2, space="PSUM") as mgpool,
        ):
            # ---- constants ----
            emb_sb = cpool.tile([P, 2, NE], dt.float16)
            nc.sync.dma_start(emb_sb[:], embed16.rearrange("(o p) c -> p o c", p=P))
            h16b = cpool.tile([P, NCHUNK, CHUNK], dt.float16)
            nc.sync.dma_start(h16b[:], h16b_d.rearrange("p (c k) -> p c k", c=NCHUNK))
            ones128 = cpool.tile([P, 1], dt.float16)
            nc.vector.memset(ones128[:], 1.0)
            ident = cpool.tile([P, P], dt.float32)
            make_identity(nc, ident[:])
            coli = cpool.tile([P, P], dt.int16)
            nc.gpsimd.iota(coli[:], pattern=[[1, P]], base=0, channel_multiplier=0)
            rowi = cpool.tile([P, 1], dt.int16)
            nc.gpsimd.iota(rowi[:], pattern=[[0, 1]], base=0, channel_multiplier=1)
            rowf = cpool.tile([P, 1], dt.float32)
            nc.vector.tensor_copy(rowf[:], rowi[:])
            ltri = cpool.tile([P, P], dt.float16)
            nc.vector.tensor_scalar(ltri[:], coli[:], rowf[:, 0:1], None, op0=AO.is_gt)
            offs = cpool.tile([P, 4], dt.uint16)
            for j in range(4):
                nc.vector.memset(offs[:, j:j + 1], j * 128)
            # zero both slabs
            zsb = cpool.tile([P, 512], dt.float32)
            nc.vector.memset(zsb[:], 0.0)
            for slab in (slab0, slab1):
                flat_slab = slab.rearrange("a b -> (a b)")
                total = (NE + 1) * DIM
                off = 0
                while off < total:
                    n = min(P * 512, total - off)
                    rows = max(n // 512, 1)
                    w = min(512, total - off)
                    nc.sync.dma_start(
                        flat_slab[off:off + rows * w].rearrange("(p f) -> p f", p=rows),
                        zsb[:rows, :w])
                    off += rows * w

            for st in range(nst):
                base = st * ST * P
                # ---- batched x load / cast (hoisted priority for prefetch) ----
                with tc.high_priority():
                    xld = xpool.tile([P, ST, DIM], dt.float32, tag="xld")
                    nc.sync.dma_start(
                        xld[:], xpart[base:base + ST * P, :].rearrange("(s p) d -> p s d", p=P))
                    x16b = xpool.tile([P, ST, DIM], dt.float16, tag="x16b")
                    nc.vector.tensor_copy(x16b[:], xld[:])
                    xTs = []
                    for s in range(ST):
                        xT = xpool.tile([P, 2, P], dt.float16, tag=f"xT{s}")
                        for k in range(2):
                            nc.sync.dma_start_transpose(xT[:, k, :], x16b[:, s, k * P:(k + 1) * P])
                        xTs.append(xT)

                mxST = wpool.tile([P, ST, 8], dt.float16, tag="mxST")
                miST = wpool.tile([P, ST, 8], dt.uint16, tag="miST")
                for s in range(ST):
                    xT = xTs[s]
                    # ---- dist matmuls + bias; evac to fp16 ----
                    s16 = bpool.tile([P, NCHUNK, CHUNK], dt.float16, tag="s16")
                    for g in range(8):
                        ps = pspool.tile([P, 2 * CHUNK], dt.float32, tag="dist")
                        for jj in range(2):
                            j = 2 * g + jj
                            sl = ps[:, jj * CHUNK:(jj + 1) * CHUNK]
                            nc.tensor.matmul(sl, xT[:, 0, :], emb_sb[:, 0, j * CHUNK:(j + 1) * CHUNK], start=True, stop=False)
                            nc.tensor.matmul(sl, xT[:, 1, :], emb_sb[:, 1, j * CHUNK:(j + 1) * CHUNK], start=False, stop=True)
                        nc.any.tensor_copy(
                            s16[:, 2 * g:2 * g + 2, :].rearrange("p a b -> p (a b)"), ps[:])
                    # ---- h subtract (in place) + folds ----
                    s16v = s16[:].rearrange("p c (two k) -> p c two k", two=2)
                    h16v = h16b[:].rearrange("p c (two k) -> p c two k", two=2)
                    nc.vector.tensor_tensor(s16v[:, :, 0, :], s16v[:, :, 0, :], h16v[:, :, 0, :], op=AO.subtract)
                    nc.vector.tensor_tensor(s16v[:, :, 1, :], s16v[:, :, 1, :], h16v[:, :, 1, :], op=AO.subtract)
                    f1 = bpool.tile([P, NCHUNK, 256], dt.float16, tag="f1")
                    nc.vector.tensor_tensor(f1[:], s16v[:, :, 0, :], s16v[:, :, 1, :], op=AO.max)
                    f1v = f1[:].rearrange("p c (two k) -> p c two k", two=2)
                    # fold2 into the (now dead) first quarter of s16 - contiguous
                    f2s = s16[:, 0:4, :].rearrange("p a b -> p (a b)")
                    nc.vector.tensor_tensor(f2s, f1v[:, :, 0, :], f1v[:, :, 1, :], op=AO.max)
                    nc.vector.max(mxST[:, s, :], f2s)
                    nc.vector.max_index(miST[:, s, :], mxST[:, s, :], f2s)

                # ---- batched candidate expansion ----
                bq = wpool.tile([P, ST, 2], dt.uint16, tag="bq")
                mp = miST[:, :, 0:2]
                nc.vector.tensor_scalar(bq[:], mp, 7, None, op0=AO.logical_shift_right)
                nc.vector.tensor_scalar(bq[:], bq[:], 9, None, op0=AO.logical_shift_left)
                kq = wpool.tile([P, ST, 2], dt.uint16, tag="kq")
                nc.vector.tensor_scalar(kq[:], mp, 127, None, op0=AO.bitwise_and)
                nc.vector.tensor_tensor(bq[:], bq[:], kq[:], op=AO.add)
                candu = wpool.tile([P, ST, 2, 4], dt.uint16, tag="candu")
                nc.vector.tensor_tensor(
                    candu[:], bq[:, :, :, None].to_broadcast([P, ST, 2, 4]),
                    offs[:, None, None, :].to_broadcast([P, ST, 2, 4]), op=AO.add)
                candf = wpool.tile([P, ST, 8], dt.float32, tag="candf")
                cu3 = candu[:].rearrange("p s a b -> p s (a b)")
                nc.vector.tensor_copy(candf[:], cu3)
                cand32 = wpool.tile([P, ST, 8], dt.int32, tag="cand32")
                nc.vector.tensor_copy(cand32[:], cu3)

                # ---- rescore ----
                x2 = wpool.tile([P, ST, DIM], dt.float32, tag="x2")
                nc.vector.tensor_scalar(x2[:], xld[:], 2.0, None, op0=AO.mult)
                dsumST = wpool.tile([P, ST, 8], dt.float32, tag="dsumST")
                t1ST = wpool.tile([P, ST], dt.float32, tag="t1ST")
                for s in range(ST):
                    G = bpool.tile([P, 8, DIM], dt.float32, tag="G")
                    for j in range(8):
                        nc.gpsimd.indirect_dma_start(
                            out=G[:, j, :], out_offset=None, in_=etab[:],
                            in_offset=IOff(ap=cand32[:, s, j:j + 1], axis=0))
                    e2x = bpool.tile([P, 8, DIM], dt.float32, tag="e2x")
                    nc.gpsimd.tensor_tensor(
                        e2x[:], G[:], x2[:, s, None, :].to_broadcast([P, 8, DIM]), op=AO.subtract)
                    nc.gpsimd.tensor_tensor(e2x[:], G[:], e2x[:], op=AO.mult)
                    nc.vector.tensor_reduce(dsumST[:, s, :], e2x[:], axis=AX.X, op=AO.add)
                    xsqj = wpool.tile([P, DIM], dt.float32, tag="xsqj")
                    nc.scalar.activation(xsqj[:], xld[:, s, :], AF.Square,
                                         accum_out=t1ST[:, s:s + 1])

                dfin = wpool.tile([P, ST, 8], dt.float32, tag="dfin")
                nc.vector.tensor_tensor(
                    dfin[:], dsumST[:], t1ST[:, :, None].to_broadcast([P, ST, 8]), op=AO.add)
                dmin = wpool.tile([P, ST], dt.float32, tag="dmin")
                nc.vector.tensor_reduce(dmin[:], dfin[:], axis=AX.X, op=AO.min)
                mask = wpool.tile([P, ST, 8], dt.float32, tag="mask")
                nc.vector.tensor_tensor(
                    mask[:], dfin[:], dmin[:, :, None].to_broadcast([P, ST, 8]), op=AO.is_equal)
                crev = wpool.tile([P, ST, 8], dt.float32, tag="crev")
                nc.vector.scalar_tensor_tensor(crev[:], candf[:], -1.0, mask[:], op0=AO.mult, op1=AO.mult)
                enc = wpool.tile([P, ST, 8], dt.float32, tag="enc")
                nc.vector.scalar_tensor_tensor(enc[:], mask[:], 8192.0, crev[:], op0=AO.mult, op1=AO.add)
                emax = wpool.tile([P, ST], dt.float32, tag="emax")
                nc.vector.tensor_reduce(emax[:], enc[:], axis=AX.X, op=AO.max)
                indf = wpool.tile([P, ST], dt.float32, tag="indf")
                nc.vector.tensor_scalar(indf[:], emax[:], -1.0, None, op0=AO.mult)
                nc.vector.tensor_scalar(indf[:], indf[:], 8192.0, None, op0=AO.add)
                dbig = wpool.tile([P, ST, 8], dt.float32, tag="dbig")
                nc.vector.scalar_tensor_tensor(dbig[:], mask[:], 1e9, dfin[:], op0=AO.mult, op1=AO.add)
                d2 = wpool.tile([P, ST], dt.float32, tag="d2")
                nc.vector.tensor_reduce(d2[:], dbig[:], axis=AX.X, op=AO.min)
                ind32 = wpool.tile([P, ST], dt.int32, tag="ind32")
                nc.vector.tensor_copy(ind32[:], indf[:])

                # ---- quantize gather + straight-through (batched out) ----
                qg = wpool.tile([P, ST, DIM], dt.float32, tag="qg")
                for s in range(ST):
                    nc.gpsimd.indirect_dma_start(
                        out=qg[:, s, :], out_offset=None, in_=etab[:],
                        in_offset=IOff(ap=ind32[:, s:s + 1], axis=0))
                qd = wpool.tile([P, ST, DIM], dt.float32, tag="qd")
                nc.vector.tensor_tensor(qd[:], qg[:], xld[:], op=AO.subtract)
                nc.vector.tensor_tensor(qd[:], xld[:], qd[:], op=AO.add)
                nc.sync.dma_start(
                    qst_o[base:base + ST * P, :].rearrange("(s p) d -> p s d", p=P), qd[:])

                # ---- duplicate merge + scatter ----
                fmaskST = wpool.tile([P, ST], dt.float32, tag="fmaskST")
                scat_in = wpool.tile([P, ST, DIM], dt.float32, tag="scatin")
                for s in range(ST):
                    ind_bc = indf[:, s:s + 1].to_broadcast([P, P])
                    mgps = mgpool.tile([P, CHUNK], dt.float32, tag="mg")
                    nc.tensor.transpose(mgps[:, 0:P], ind_bc, ident[:])
                    indT = wpool.tile([P, P], dt.float32, tag="indT")
                    nc.any.tensor_copy(indT[:], mgps[:, 0:P])
                    sel16 = wpool.tile([P, P], dt.float16, tag="sel16")
                    nc.vector.tensor_tensor(sel16[:], ind_bc, indT[:], op=AO.is_equal)
                    lsel = wpool.tile([P, P], dt.float16, tag="lsel")
                    nc.vector.tensor_tensor(lsel[:], sel16[:], ltri[:], op=AO.mult)
                    nc.tensor.matmul(mgps[:, 384:385], lsel[:], ones128[:], start=True, stop=True)
                    nc.vector.tensor_scalar(fmaskST[:, s:s + 1], mgps[:, 384:385], 0.0, None, op0=AO.is_equal)
                    nc.tensor.matmul(mgps[:, P:P + DIM], sel16[:], x16b[:, s, :], start=True, stop=True)
                    nc.scalar.activation(scat_in[:, s, :], mgps[:, P:P + DIM],
                                         AF.Copy, scale=fmaskST[:, s:s + 1])
                sidxf = wpool.tile([P, ST], dt.float32, tag="sidxf")
                nc.vector.tensor_scalar(sidxf[:], indf[:], 8192.0, None, op0=AO.subtract)
                nc.vector.tensor_tensor(sidxf[:], sidxf[:], fmaskST[:], op=AO.mult)
                nc.vector.tensor_scalar(sidxf[:], sidxf[:], 8192.0, None, op0=AO.add)
                sidx32 = wpool.tile([P, ST], dt.int32, tag="sidx32")
                nc.vector.tensor_copy(sidx32[:], sidxf[:])
                for s in range(ST):
                    slab = slab0 if (s % 2 == 0) else slab1
                    nc.gpsimd.indirect_dma_start(
                        out=slab[:], out_offset=IOff(ap=sidx32[:, s:s + 1], axis=0),
                        in_=scat_in[:, s, :], in_offset=None, compute_op=AO.add)

                # ---- aux out ----
                auxsb = wpool.tile([P, ST, 4], dt.float32, tag="auxsb")
                nc.vector.tensor_copy(auxsb[:, :, 0], indf[:])
                nc.vector.tensor_copy(auxsb[:, :, 1], dmin[:])
                nc.vector.tensor_copy(auxsb[:, :, 2], d2[:])
                nc.vector.tensor_copy(auxsb[:, :, 3], t1ST[:])
                nc.sync.dma_start(
                    aux[base:base + ST * P, :].rearrange("(s p) c -> p s c", p=P), auxsb[:])

            # ---- merge slab1 into slab0 via DMA accumulate ----
            s0f = slab0.rearrange("a b -> (a b)")
            s1f = slab1.rearrange("a b -> (a b)")
            blk = P * 512
            pieces = [(i * blk, P, 512) for i in range(NE * DIM // blk)]
            pieces.append((NE * DIM, 1, DIM))  # dummy row (not needed, but cheap)
            for off, rows, w in pieces:
                mt = wpool.tile([P, 512], dt.float32, tag="mrg")
                nc.sync.dma_start(
                    mt[:rows, :w],
                    s1f[off:off + rows * w].rearrange("(p f) -> p f", p=rows))
                nc.gpsimd.dma_start(
                    s0f[off:off + rows * w].rearrange("(p f) -> p f", p=rows),
                    mt[:rows, :w], accum_op=AO.add)

    nc.compile()
    return nc


def _get_nc(nst=NST):
    key = ("nc", nst)
    if key not in _STATE:
        _STATE[key] = _build(nst)
    return _STATE[key]


def _host_prep(x, embed):
    flat = np.ascontiguousarray(x.reshape(-1, DIM)).astype(np.float32, copy=False)
    embed = np.asarray(embed, np.float32)
    embed16 = embed.astype(np.float16)
    h = (embed.astype(np.float32) ** 2).sum(0, dtype=np.float32)
    h16b = np.ascontiguousarray(np.broadcast_to((h / 2.0).astype(np.float16)[None, :], (P, NE)))
    etab = np.ascontiguousarray(embed.T)
    return flat, embed16, h16b, etab, h


def kernel(x, embed, cluster_size, embed_avg):
    from concourse import bass_utils

    x = np.asarray(x, np.float32)
    embed = np.asarray(embed, np.float32)
    cluster_size = np.asarray(cluster_size, np.float32)
    embed_avg = np.asarray(embed_avg, np.float32)

    flat, embed16, h16b, etab, h = _host_prep(x, embed)
    N = flat.shape[0]
    assert N == N_CORES * TPC

    nc = _get_nc()
    in_maps = []
    for c in range(N_CORES):
        in_maps.append({
            "xpart": flat[c * TPC:(c + 1) * TPC],
            "embed16": embed16,
            "h16b": h16b,
            "etab": etab,
        })
    res = bass_utils.run_bass_kernel_spmd(nc, in_maps, core_ids=list(range(N_CORES)))

    aux = np.concatenate([res.results[c]["aux"] for c in range(N_CORES)], axis=0)
    qst = np.concatenate([res.results[c]["qst"] for c in range(N_CORES)], axis=0)
    embed_sum = np.zeros((NE, DIM), np.float32)
    for c in range(N_CORES):
        embed_sum += res.results[c]["slab0"][:NE]
    embed_sum = embed_sum.T.copy()          # [256, 8192]

    ind = aux[:, 0].astype(np.int64)
    d1 = aux[:, 1].astype(np.float64)
    d2 = aux[:, 2]

    gap = d2 - aux[:, 1]
    fix = np.nonzero(gap < GAP_THRESH)[0]
    if fix.size:
        import jax
        import jax.numpy as jnp
        cpu = jax.devices("cpu")[0]
        with jax.default_device(cpu):
            f = jnp.asarray(flat[fix])
            e = jnp.asarray(embed)
            dist = (jnp.sum(f * f, axis=1, keepdims=True) - 2.0 * (f @ e)
                    + jnp.sum(e * e, axis=0, keepdims=True))
            new_ind = np.asarray(jnp.argmin(dist, axis=1)).astype(np.int64)
            dist = np.asarray(dist)
        d1[fix] = dist[np.arange(fix.size), new_ind]
        changed_mask = new_ind != ind[fix]
        for tk, nw in zip(fix[changed_mask], new_ind[changed_mask]):
            od = int(ind[tk])
            ind[tk] = nw
            q_new = embed[:, nw]
            qst[tk] = flat[tk] + (q_new - flat[tk])
            embed_sum[:, od] -= flat[tk]
            embed_sum[:, nw] += flat[tk]

    counts = np.bincount(ind, minlength=NE).astype(np.float32)
    cluster_size_new = (cluster_size * np.float32(DECAY)
                        + np.float32(1.0 - DECAY) * counts).astype(np.float32)
    embed_avg_new = (embed_avg * np.float32(DECAY)
                     + np.float32(1.0 - DECAY) * embed_sum).astype(np.float32)
    n = np.float32(cluster_size_new.sum(dtype=np.float64))
    cs = (cluster_size_new + np.float32(EPS)) / (n + np.float32(NE * EPS)) * n
    embed_new = (embed_avg_new / cs[None, :]).astype(np.float32)

    diff = np.float32(d1.sum() / (N * DIM))
    quantize_st = qst.reshape(x.shape).astype(np.float32)
    embed_ind = ind.astype(np.int32).reshape(x.shape[:-1])

    return (quantize_st, diff, embed_ind, embed_new,
            cluster_size_new, embed_avg_new)
